# revision 10
# baseline (speedup 1.0000x reference)
"""CapsNet forward kernel for 8 Trainium2 NeuronCores (Bass/Tile).

Sharding: pure data parallelism. Each core processes 32 of 256 images;
weights are replicated. Host gathers per-core (v, probs) shards.

Per-core pipeline:
  conv1 (PE, im2col K=81)  -> x (128ch-half, 32b, 20, 20) fp32, one ic-half
                              at a time (SBUF reuse)
  conv2 (PE, 162 K-steps accumulated in PSUM) -> y (256ch, 32b, 36) + bias
  capsule predictions (PE): per (g, s) matmul  u_hat = y[g,s,:8].T @ Wcaps
     done as K=32 col-tiled matmuls with zero-padded weights so that the
     PSUM output lands directly in (4g x 32b)-partition layout.  An
     appended 8-column identity block also transposes raw y into that
     layout (used for the squash factor).
  squash is folded in algebraically: up = f * (y @ Wc) with
     f = l2/((1+l2)(sqrt(l2)+1e-8)) per (b,g,s), so f multiplies the
     softmax coefficients (E1) and the logit update (E2) instead of up.
  routing (DVE/ACT), 3 iterations, instances (b,g) on partitions.
  v = sum_g v_g via a mask matmul on PE accumulated across 8 waves.
"""

import sys

for _p in ("/opt/trn_rl_repo", "/root/.axon_site/_ro/trn_rl_repo"):
    if _p not in sys.path:
        sys.path.insert(0, _p)

import numpy as np

import concourse.bass as bass
import concourse.tile as tile
from concourse import bacc, mybir
from concourse.bass_utils import run_bass_kernel_spmd

F32 = mybir.dt.float32
AF = mybir.ActivationFunctionType
OP = mybir.AluOpType
AX = mybir.AxisListType

N_CORES = 8
B = 256
NB = B // N_CORES  # 32 images per core
G, PD, OC, OD, GS = 32, 8, 10, 16, 36
NI = 3
ODI = OC * OD  # 160
ODE = ODI + PD  # 168: caps columns + identity block for raw-y transpose
SH = GS // 2  # 18, s-half for routing temporaries
SQ = GS // 4  # 9, s-quarter for padded caps weights
# conv2 batch sub-chunks (PSUM free-dim <= 512 fp32)
SUBS = [(0, 11), (11, 11), (22, 10)]


def _v(ap, dims, off=0):
    """Free-dim view of an SBUF AP: keep its partition dim, replace free
    dims with explicit [step, count] pairs (steps in elements)."""
    return bass.AP(
        tensor=ap.tensor,
        offset=ap.offset + off,
        ap=[list(ap.ap[0])] + [[s, c] for s, c in dims],
    )


def _dv(ap, dims, off=0):
    """Raw multi-dim view of a DRAM AP (no partition dim)."""
    return bass.AP(
        tensor=ap.tensor,
        offset=ap.offset + off,
        ap=[[s, c] for s, c in dims],
    )


def build_nc():
    nc = bacc.Bacc(
        "TRN2", target_bir_lowering=False, debug=False, num_devices=N_CORES
    )

    inp = nc.dram_tensor("inp", [NB, 81, 400], F32, kind="ExternalInput").ap()
    w1t = nc.dram_tensor("w1t", [81, 256], F32, kind="ExternalInput").ap()
    b1c = nc.dram_tensor("b1c", [128, 2], F32, kind="ExternalInput").ap()
    w2t = nc.dram_tensor(
        "w2t", [2, 9, 9, 2, 128, 128], F32, kind="ExternalInput"
    ).ap()
    b2c = nc.dram_tensor("b2c", [128, 2], F32, kind="ExternalInput").ap()
    wce = nc.dram_tensor("wce", [G, GS, PD, ODE], F32, kind="ExternalInput").ap()
    br = nc.dram_tensor("br", [G, GS * OC], F32, kind="ExternalInput").ap()
    vmask = nc.dram_tensor("vmask", [128, 32], F32, kind="ExternalInput").ap()

    v_out = nc.dram_tensor("v", [NB, OC, OD], F32, kind="ExternalOutput").ap()
    p_out = nc.dram_tensor("probs", [NB, OC], F32, kind="ExternalOutput").ap()

    with tile.TileContext(nc) as tc:
        with (
            tc.tile_pool(name="const", bufs=1) as constp,
            tc.tile_pool(name="xbuf", bufs=1) as xpool,
            tc.tile_pool(name="ybuf", bufs=1) as ypool,
            tc.tile_pool(name="wc", bufs=1) as wcpool,
            tc.tile_pool(name="upbuf", bufs=1) as uppool,
            tc.tile_pool(name="ywm", bufs=1) as ywmpool,
            tc.tile_pool(name="tmp", bufs=2) as tmppool,
            tc.tile_pool(name="rt", bufs=2) as rt,
            tc.tile_pool(name="psv", bufs=1, space="PSUM") as psvpool,
        ):
            # ---- constants -------------------------------------------------
            w1s = constp.tile([81, 256], F32)
            nc.sync.dma_start(out=w1s[:], in_=w1t)
            b1s = constp.tile([128, 2], F32)
            nc.sync.dma_start(out=b1s[:], in_=b1c)
            b2s = constp.tile([128, 2], F32)
            nc.sync.dma_start(out=b2s[:], in_=b2c)
            vms = constp.tile([128, 32], F32)
            nc.sync.dma_start(out=vms[:], in_=vmask)

            # persistent zero-padded caps weights (two s-quarter slots)
            wcp = wcpool.tile([32, 4, SQ, ODE], F32, name="wcp0", tag="wcp0")
            nc.gpsimd.memset(wcp[:], 0.0)

            x_t = xpool.tile([128, NB, 20, 20], F32)
            y_t = [ypool.tile([128, NB, GS], F32, name=f"y{h}", tag=f"y{h}") for h in (0, 1)]

            # ---- conv1 + conv2 --------------------------------------------
            with (
                tc.tile_pool(name="im", bufs=2) as impool,
                tc.tile_pool(name="w2b", bufs=3) as w2pool,
                tc.tile_pool(name="pc1", bufs=1, space="PSUM") as pc1,
                tc.tile_pool(name="pc2", bufs=1, space="PSUM") as pc2,
            ):
                pconv2 = [
                    [pc2.tile([128, n * GS], F32, name=f"p2_{mh}_{si}", tag=f"p2_{mh}_{si}") for si, (_, n) in enumerate(SUBS)]
                    for mh in (0, 1)
                ]
                xr = x_t[:].rearrange(
                    "p b (h a) (w c) -> p b h a w c", a=2, c=2
                )  # (128, NB, 10, 2, 10, 2)
                for ic in (0, 1):
                    # conv1 for this input-channel half of conv2
                    for bc in range(NB // 4):
                        im = impool.tile([81, 4, 400], F32)
                        src = _dv(
                            inp,
                            [[400, 81], [81 * 400, 4], [1, 400]],
                            off=bc * 4 * 81 * 400,
                        )
                        nc.sync.dma_start(out=im[:], in_=src)
                        for i in range(4):
                            ps1 = pc1.tile([128, 400], F32)
                            nc.tensor.matmul(
                                ps1[:],
                                w1s[:, ic * 128 : ic * 128 + 128],
                                im[:, i, :],
                                start=True,
                                stop=True,
                            )
                            nc.scalar.activation(
                                out=_v(x_t[:], [[1, 400]], off=(bc * 4 + i) * 400),
                                in_=ps1[:],
                                func=AF.Relu,
                                bias=b1s[:, ic : ic + 1],
                                scale=1.0,
                            )
                    # conv2 accumulation for this ic half
                    for dy in range(9):
                        qy, ry = dy // 2, dy % 2
                        for dx in range(9):
                            qx, rx = dx // 2, dx % 2
                            first = ic == 0 and dy == 0 and dx == 0
                            last = ic == 1 and dy == 8 and dx == 8
                            for mh in (0, 1):
                                w2 = w2pool.tile([128, 128], F32)
                                nc.sync.dma_start(
                                    out=w2[:], in_=w2t[ic, dy, dx, mh]
                                )
                                for si, (b0, nbs) in enumerate(SUBS):
                                    rhs = xr[
                                        :, b0 : b0 + nbs, qy : qy + 6, ry, qx : qx + 6, rx
                                    ]
                                    nc.tensor.matmul(
                                        pconv2[mh][si][:],
                                        w2[:],
                                        rhs,
                                        start=first,
                                        stop=last,
                                    )
                for mh in (0, 1):
                    for si, (b0, nbs) in enumerate(SUBS):
                        nc.vector.tensor_scalar_add(
                            y_t[mh][:, b0 : b0 + nbs, :],
                            pconv2[mh][si][:].rearrange(
                                "p (b s) -> p b s", s=GS
                            ),
                            b2s[:, mh : mh + 1],
                        )

            # ---- waves: caps-pred + routing -------------------------------
            psv = psvpool.tile([32, ODI], F32)
            with tc.tile_pool(name="pcap", bufs=4, space="PSUM") as pcaps:
                for w in range(8):
                    h, vv = w // 4, w % 4
                    g0 = 16 * h + 4 * vv
                    # y wave slice, replicated x4 in free dim for col-tiling
                    ywm = ywmpool.tile([32, 4, NB, GS], F32)
                    ysl = y_t[h][32 * vv : 32 * vv + 32, :, :]
                    nc.gpsimd.dma_start(
                        out=ywm[:],
                        in_=_v(ysl, [[0, 4], [GS, NB], [1, GS]]),
                    )
                    up_a = uppool.tile([128, OC, OD, GS], F32)
                    yhat = rt.tile([128, GS, PD], F32, tag="yhat")
                    wcer = wce.rearrange("g s i c -> g i s c")
                    for q in range(4):
                        wq = wcp
                        for j in range(4):
                            nc.sync.dma_start(
                                out=wq[8 * j : 8 * j + 8, j, :, :],
                                in_=wcer[g0 + j, :, SQ * q : SQ * q + SQ, :],
                            )
                        for sq_i in range(SQ):
                            s = SQ * q + sq_i
                            pc = pcaps.tile([128, ODE], F32, tag="pcap")
                            for j in range(4):
                                nc.tensor.matmul(
                                    pc[32 * j : 32 * j + 32, :],
                                    ywm[:, j, :, s],
                                    wq[:, j, sq_i, :],
                                    start=True,
                                    stop=True,
                                    tile_position=(0, 32 * j),
                                )
                            # scatter psum -> up_a[:, :, :, s] and yhat
                            nc.scalar.copy(
                                out=_v(
                                    up_a[:],
                                    [[OD * GS, OC], [GS, OD]],
                                    off=s,
                                ),
                                in_=pc[:, 0:ODI],
                            )
                            nc.scalar.copy(
                                out=yhat[:, s, :],
                                in_=pc[:, ODI:ODE],
                            )
                    # squash factor f per (partition=(g,b), s)
                    sqy = rt.tile([128, GS, PD], F32, tag="sqy")
                    nc.vector.tensor_mul(sqy[:], yhat[:], yhat[:])
                    l2 = rt.tile([128, GS], F32, tag="l2")
                    nc.vector.reduce_sum(l2[:], sqy[:], axis=AX.X)
                    l2g = rt.tile([128, GS], F32, tag="l2g")
                    nc.vector.tensor_scalar_add(l2g[:], l2[:], 1e-35)
                    lf = rt.tile([128, GS], F32, tag="lf")
                    nc.scalar.activation(out=lf[:], in_=l2g[:], func=AF.Sqrt)
                    # one Newton step for sqrt accuracy
                    rl = rt.tile([128, GS], F32, tag="rl")
                    nc.vector.reciprocal(rl[:], lf[:])
                    t0 = rt.tile([128, GS], F32, tag="t0")
                    nc.vector.tensor_mul(t0[:], l2g[:], rl[:])
                    t1 = rt.tile([128, GS], F32, tag="t1")
                    nc.vector.tensor_add(t1[:], lf[:], t0[:])
                    lfr = rt.tile([128, GS], F32, tag="lfr")
                    nc.vector.tensor_scalar_mul(lfr[:], t1[:], 0.5)
                    d0 = rt.tile([128, GS], F32, tag="d0")
                    nc.vector.tensor_scalar_add(d0[:], l2[:], 1.0)
                    d1 = rt.tile([128, GS], F32, tag="d1")
                    nc.vector.tensor_scalar_add(d1[:], lfr[:], 1e-8)
                    d2 = rt.tile([128, GS], F32, tag="d2")
                    nc.vector.tensor_mul(d2[:], d0[:], d1[:])
                    rd = rt.tile([128, GS], F32, tag="rd")
                    nc.vector.reciprocal(rd[:], d2[:])
                    fs = rt.tile([128, GS], F32, tag="fs")
                    nc.vector.tensor_mul(fs[:], l2[:], rd[:])

                    # routing logits init: broadcast b_route over batch
                    bt = rt.tile([128, GS, OC], F32, tag="bt")
                    nc.gpsimd.dma_start(
                        out=bt[:],
                        in_=_dv(
                            br,
                            [[GS * OC, 4], [0, 32], [1, GS * OC]],
                            off=g0 * GS * OC,
                        ),
                    )
                    vt = None
                    for r in range(NI):
                        e = rt.tile([128, GS, OC], F32, tag="e")
                        nc.scalar.activation(out=e[:], in_=bt[:], func=AF.Exp)
                        se = rt.tile([128, GS], F32, tag="se")
                        nc.vector.reduce_sum(se[:], e[:], axis=AX.X)
                        rinv = rt.tile([128, GS], F32, tag="rinv")
                        nc.vector.reciprocal(rinv[:], se[:])
                        rf = rt.tile([128, GS], F32, tag="rf")
                        nc.vector.tensor_mul(rf[:], rinv[:], fs[:])
                        ct = rt.tile([128, OC, GS], F32, tag="ct")
                        nc.vector.tensor_mul(
                            _v(ct[:], [[1, GS], [GS, OC]]),
                            e[:],
                            _v(rf[:], [[1, GS], [0, OC]]),
                        )
                        sacc = rt.tile([128, OC, OD], F32, tag="sacc")
                        sach = rt.tile([128, 2, OC, OD], F32, tag="sach")
                        for hf in (0, 1):
                            tmp = tmppool.tile([128, OC, OD, SH], F32, tag="tmp")
                            nc.vector.tensor_mul(
                                tmp[:],
                                up_a[:, :, :, SH * hf : SH * hf + SH],
                                _v(
                                    ct[:],
                                    [[GS, OC], [0, OD], [1, SH]],
                                    off=SH * hf,
                                ),
                            )
                            nc.vector.reduce_sum(
                                sach[:, hf, :, :], tmp[:], axis=AX.X
                            )
                        nc.vector.tensor_add(
                            sacc[:], sach[:, 0, :, :], sach[:, 1, :, :]
                        )
                        # v = squash(sacc)
                        sqs = rt.tile([128, OC, OD], F32, tag="sqs")
                        nc.vector.tensor_mul(sqs[:], sacc[:], sacc[:])
                        l2v = rt.tile([128, OC], F32, tag="l2v")
                        nc.vector.reduce_sum(l2v[:], sqs[:], axis=AX.X)
                        l2vg = rt.tile([128, OC], F32, tag="l2vg")
                        nc.vector.tensor_scalar_add(l2vg[:], l2v[:], 1e-35)
                        lv = rt.tile([128, OC], F32, tag="lv")
                        nc.scalar.activation(out=lv[:], in_=l2vg[:], func=AF.Sqrt)
                        rlv = rt.tile([128, OC], F32, tag="rlv")
                        nc.vector.reciprocal(rlv[:], lv[:])
                        tv0 = rt.tile([128, OC], F32, tag="tv0")
                        nc.vector.tensor_mul(tv0[:], l2vg[:], rlv[:])
                        tv1 = rt.tile([128, OC], F32, tag="tv1")
                        nc.vector.tensor_add(tv1[:], lv[:], tv0[:])
                        lvr = rt.tile([128, OC], F32, tag="lvr")
                        nc.vector.tensor_scalar_mul(lvr[:], tv1[:], 0.5)
                        dv0 = rt.tile([128, OC], F32, tag="dv0")
                        nc.vector.tensor_scalar_add(dv0[:], l2v[:], 1.0)
                        dv1 = rt.tile([128, OC], F32, tag="dv1")
                        nc.vector.tensor_scalar_add(dv1[:], lvr[:], 1e-8)
                        dv2 = rt.tile([128, OC], F32, tag="dv2")
                        nc.vector.tensor_mul(dv2[:], dv0[:], dv1[:])
                        rdv = rt.tile([128, OC], F32, tag="rdv")
                        nc.vector.reciprocal(rdv[:], dv2[:])
                        fv = rt.tile([128, OC], F32, tag="fv")
                        nc.vector.tensor_mul(fv[:], l2v[:], rdv[:])
                        vt = rt.tile([128, OC, OD], F32, tag="vt")
                        nc.vector.tensor_mul(
                            vt[:],
                            sacc[:],
                            _v(fv[:], [[1, OC], [0, OD]]),
                        )
                        if r < NI - 1:
                            db = rt.tile([128, OC, GS], F32, tag="db")
                            for hf in (0, 1):
                                tmp2 = tmppool.tile(
                                    [128, OC, SH, OD], F32, tag="tmp"
                                )
                                nc.vector.tensor_mul(
                                    tmp2[:],
                                    _v(
                                        up_a[:],
                                        [[OD * GS, OC], [1, SH], [GS, OD]],
                                        off=SH * hf,
                                    ),
                                    _v(vt[:], [[OD, OC], [0, SH], [1, OD]]),
                                )
                                nc.vector.reduce_sum(
                                    _v(
                                        db[:],
                                        [[GS, OC], [1, SH]],
                                        off=SH * hf,
                                    ),
                                    tmp2[:],
                                    axis=AX.X,
                                )
                            dbf = rt.tile([128, OC, GS], F32, tag="dbf")
                            nc.vector.tensor_mul(
                                dbf[:],
                                db[:],
                                _v(fs[:], [[0, OC], [1, GS]]),
                            )
                            btn = rt.tile([128, GS, OC], F32, tag="bt")
                            nc.vector.tensor_add(
                                btn[:],
                                bt[:],
                                _v(dbf[:], [[1, GS], [GS, OC]]),
                            )
                            bt = btn
                    # accumulate v over groups: psv[b, :] += sum_j vt[32j+b, :]
                    nc.tensor.matmul(
                        psv[:],
                        vms[:],
                        vt[:].rearrange("p o d -> p (o d)"),
                        start=(w == 0),
                        stop=(w == 7),
                        skip_group_check=True,
                    )

            # ---- finals ----------------------------------------------------
            vo = rt.tile([32, OC, OD], F32, tag="vo")
            nc.scalar.copy(
                out=vo[:].rearrange("p o d -> p (o d)"), in_=psv[:]
            )
            sqv = rt.tile([32, OC, OD], F32, tag="sqv")
            nc.vector.tensor_mul(sqv[:], vo[:], vo[:])
            p2 = rt.tile([32, OC], F32, tag="p2")
            nc.vector.reduce_sum(p2[:], sqv[:], axis=AX.X)
            p2g = rt.tile([32, OC], F32, tag="p2g")
            nc.vector.tensor_scalar_add(p2g[:], p2[:], 1e-35)
            pr = rt.tile([32, OC], F32, tag="pr")
            nc.scalar.activation(out=pr[:], in_=p2g[:], func=AF.Sqrt)
            rpr = rt.tile([32, OC], F32, tag="rpr")
            nc.vector.reciprocal(rpr[:], pr[:])
            tp0 = rt.tile([32, OC], F32, tag="tp0")
            nc.vector.tensor_mul(tp0[:], p2g[:], rpr[:])
            tp1 = rt.tile([32, OC], F32, tag="tp1")
            nc.vector.tensor_add(tp1[:], pr[:], tp0[:])
            prf = rt.tile([32, OC], F32, tag="prf")
            nc.vector.tensor_scalar_mul(prf[:], tp1[:], 0.5)

            nc.sync.dma_start(out=v_out, in_=vo[:])
            nc.sync.dma_start(out=p_out, in_=prf[:])

    nc.compile()
    return nc


def prep_host_inputs(input, W1, b1, W2, b2, Wcaps, b_route):
    """Transform full inputs into the per-core bass-kernel input dicts."""
    input = np.ascontiguousarray(input, dtype=np.float32)
    W1 = np.asarray(W1, dtype=np.float32)
    b1 = np.asarray(b1, dtype=np.float32)
    W2 = np.asarray(W2, dtype=np.float32)
    b2 = np.asarray(b2, dtype=np.float32)
    Wcaps = np.asarray(Wcaps, dtype=np.float32)
    b_route = np.asarray(b_route, dtype=np.float32)

    w1t = np.ascontiguousarray(W1.reshape(256, 81).T)  # (81, 256)
    b1c = np.ascontiguousarray(b1.reshape(2, 128).T)  # (128, 2)
    # w2t[ic, dy, dx, mh, icl, ocl] = W2[mh*128+ocl, ic*128+icl, dy, dx]
    w2t = np.ascontiguousarray(
        W2.reshape(2, 128, 2, 128, 9, 9).transpose(3, 4, 5, 0, 1, 2)
    )  # (icb, dy, dx, mh, icl, ocl) -- wait, check below
    # W2 (256oc, 256ic, 9, 9) -> view (2mh, 128ocl, 2icb, 128icl, 9, 9)
    # transpose to (icb, dy, dx, mh, icl, ocl): axes (2, 4, 5, 0, 3, 1)
    w2t = np.ascontiguousarray(
        W2.reshape(2, 128, 2, 128, 9, 9).transpose(2, 4, 5, 0, 3, 1)
    )
    b2c = np.ascontiguousarray(b2.reshape(2, 128).T)  # (128, 2)
    wcev = np.zeros((G, GS, PD, ODE), dtype=np.float32)
    wcev[:, :, :, :ODI] = Wcaps  # (G, GS, PD, 160)
    wcev[:, :, :, ODI:] = np.eye(PD, dtype=np.float32)[None, None]
    brh = np.ascontiguousarray(
        np.broadcast_to(b_route[:, 0, :, :], (G, GS, OC)).reshape(G, GS * OC)
    )
    vmask = np.zeros((128, 32), dtype=np.float32)
    for p in range(128):
        vmask[p, p % 32] = 1.0

    shared = {
        "w1t": w1t,
        "b1c": b1c,
        "w2t": w2t,
        "b2c": b2c,
        "wce": wcev,
        "br": brh,
        "vmask": vmask,
    }
    im = np.empty((B, 81, 400), dtype=np.float32)
    for dy in range(9):
        for dx in range(9):
            im[:, dy * 9 + dx, :] = input[:, 0, dy : dy + 20, dx : dx + 20].reshape(
                B, 400
            )
    in_maps = []
    for c in range(N_CORES):
        m = dict(shared)
        m["inp"] = np.ascontiguousarray(im[c * NB : (c + 1) * NB])
        in_maps.append(m)
    return in_maps


_NC_CACHE = None


def get_nc():
    global _NC_CACHE
    if _NC_CACHE is None:
        _NC_CACHE = build_nc()
    return _NC_CACHE


def kernel(input, W1, b1, W2, b2, Wcaps, b_route):
    in_maps = prep_host_inputs(input, W1, b1, W2, b2, Wcaps, b_route)
    nc = get_nc()
    res = run_bass_kernel_spmd(nc, in_maps, core_ids=list(range(N_CORES)))
    v = np.concatenate([r["v"] for r in res.results], axis=0)
    probs = np.concatenate([r["probs"] for r in res.results], axis=0)
    return v.astype(np.float32), probs.astype(np.float32)
